# revision 12
# baseline (speedup 1.0000x reference)
"""Trainium2 Bass kernel for nn_ArthDenseCalcToDenseBlock.

The reference is a 256-step sequential scan per batch row, but the state
machine freezes at the first valid operator token (the `meet` gate), so the
whole scan collapses to closed-form masked reductions along the sequence
axis, computed per row with DVE prefix-scan instructions:

  mpre[j] = running-max of (valid-op mask)        -> first-op one-hot, met
  csuf[j] = reverse running-sum of (number mask)  -> last / 2nd-last number
                                                     one-hots via == 1 / == 2
  h0,h1 and the operator channel values are gathered with masked-sum
  accumulations; one predicated scatter writes the result back.

Data parallel over batch: 4096 rows -> 8 cores x 512 rows -> 2 halves of
[128, 2x256] merged tiles per core. trans_op is host-relayouted into 7
contiguous channel planes so every channel op is a contiguous 2D access.
Mask tensors are bf16 (0/1 and small counts are exact); trans_op values and
trans_dense stay f32 so argmax/select semantics match the reference
bit-exactly. Work is spread across DVE / GpSimd / ACT.
"""

from contextlib import ExitStack

import numpy as np

import concourse.bacc as bacc
import concourse.mybir as mybir
import concourse.tile as tile
from concourse.bass_utils import run_bass_kernel_spmd

F32 = mybir.dt.float32
BF16 = mybir.dt.bfloat16
U8 = mybir.dt.uint8
OP = mybir.AluOpType
ACTF = mybir.ActivationFunctionType

B, S, NOPS = 4096, 256, 7
NCORES = 8
BS = B // NCORES          # rows per core (512)
P = 128                   # partitions
NT = BS // P              # row-tiles per core (4)
NH = 2                    # halves per core
TPH = NT // NH            # row-tiles per half (2)
W = TPH * S               # free width of a merged half (512)


def _build_nc(sp_zero: bool):
    nc = bacc.Bacc("TRN2", target_bir_lowering=False, debug=False)

    tv_d = nc.dram_tensor("tv", [BS, S], F32, kind="ExternalInput")
    td_d = nc.dram_tensor("td", [BS, S], F32, kind="ExternalInput")
    # channel planes: op[c, row, s]
    op_d = nc.dram_tensor("op", [NOPS, BS, S], F32, kind="ExternalInput")
    fv_d = nc.dram_tensor("fv", [P, 2 * NT], F32, kind="ExternalInput")
    act_d = nc.dram_tensor("act2", [W], F32, kind="ExternalInput")

    tvo_d = nc.dram_tensor("tv_out", [BS, S], F32, kind="ExternalOutput")
    tdo_d = nc.dram_tensor("td_out", [BS, S], F32, kind="ExternalOutput")
    io_d = nc.dram_tensor("iffiv", [P * 2 * NT], F32, kind="ExternalOutput")

    with tile.TileContext(nc) as tc, ExitStack() as ctx:
        cpool = ctx.enter_context(tc.tile_pool(name="consts", bufs=1))
        io_pool = ctx.enter_context(tc.tile_pool(name="io", bufs=NH + 1))
        op_pool = ctx.enter_context(tc.tile_pool(name="op", bufs=2))
        work = ctx.enter_context(tc.tile_pool(name="work", bufs=2))
        gsc = ctx.enter_context(tc.tile_pool(name="gsc", bufs=6))
        sm = ctx.enter_context(tc.tile_pool(name="small", bufs=1))

        if not sp_zero:
            crow = cpool.tile([P, W], F32)
            nc.sync.dma_start(crow[0:1, :],
                              act_d.ap().rearrange("(o s) -> o s", o=1))
            actf = cpool.tile([P, W], F32)
            nc.gpsimd.partition_broadcast(actf[:], crow[0:1, :])
            act_bc = cpool.tile([P, W], BF16)
            nc.vector.tensor_copy(act_bc[:], actf[:])
        zero_bc = cpool.tile([P, S], F32)
        nc.vector.memset(zero_bc[:], 0.0)

        # ---- per-row gates [P, NT] (f cols 0..NT-1, g cols NT..2NT-1)
        fv = sm.tile([P, 2 * NT], F32)
        nc.sync.dma_start(fv[:], fv_d[:, :])
        fin_t = fv[:, 0:NT]
        val_t = fv[:, NT : 2 * NT]
        omf = sm.tile([P, NT], F32)   # 1 - f
        nc.gpsimd.tensor_scalar(omf[:], fin_t, -1.0, 1.0, op0=OP.mult, op1=OP.add)
        gate = sm.tile([P, NT], F32)  # (1 - f) * g
        nc.gpsimd.tensor_mul(gate[:], omf[:], val_t)

        iffiv = sm.tile([P, 2 * NT], F32)
        # batched per-core scalars (columns = row-tile index 0..NT-1)
        h0_a = sm.tile([P, NT], F32)
        h1_a = sm.tile([P, NT], F32)
        vmax_a = sm.tile([P, NT], F32)
        vc_a = [sm.tile([P, NT], F32, name=f"vc{c}") for c in range(2, 7)]
        fire_a = sm.tile([P, NT], F32)
        r_a = sm.tile([P, NT], F32)

        half_state = []

        for h in range(NH):
            rows = slice(h * TPH * P, (h + 1) * TPH * P)
            tvt = io_pool.tile([P, W], F32, tag="tvt", name=f"tvt{h}")
            tdt = io_pool.tile([P, W], F32, tag="tdt", name=f"tdt{h}")
            chs = [None] * NOPS
            qeng = {1: nc.sync, 2: nc.sync, 3: nc.scalar, 4: nc.scalar,
                    5: nc.sync, 6: nc.scalar, 0: nc.sync}
            for c in [1, 2, 3, 4, 5, 6, 0]:  # tree-feeding planes first
                cht = op_pool.tile([P, W], F32, tag=f"ch{c}", name=f"ch{c}_{h}")
                qeng[c].dma_start(
                    cht[:].rearrange("p (t s) -> p t s", t=TPH),
                    op_d[c, rows, :].rearrange("(t p) s -> p t s", p=P))
                chs[c] = cht
            nc.sync.dma_start(
                tvt[:].rearrange("p (t s) -> p t s", t=TPH),
                tv_d[rows, :].rearrange("(t p) s -> p t s", p=P))
            nc.scalar.dma_start(
                tdt[:].rearrange("p (t s) -> p t s", t=TPH),
                td_d[rows, :].rearrange("(t p) s -> p t s", p=P))

            # channel max over 1..6 via TT tree (DVE/ACT; Pool has no max)
            a1 = work.tile([P, W], F32, tag="a1", name=f"a1_{h}")
            nc.vector.tensor_max(a1[:], chs[1][:], chs[2][:])
            a2 = work.tile([P, W], F32, tag="a2", name=f"a2_{h}")
            nc.any.tensor_max(a2[:], chs[3][:], chs[4][:])
            a3 = work.tile([P, W], F32, tag="a3", name=f"a3_{h}")
            nc.any.tensor_max(a3[:], chs[5][:], chs[6][:])
            b1 = work.tile([P, W], F32, tag="b1", name=f"b1_{h}")
            nc.any.tensor_max(b1[:], a1[:], a2[:])
            m6 = work.tile([P, W], F32, tag="m6", name=f"m6_{h}")
            nc.vector.tensor_max(m6[:], b1[:], a3[:])
            is_op = work.tile([P, W], BF16, tag="isop", name=f"isop_{h}")
            nc.any.tensor_tensor(is_op[:], m6[:], chs[0][:], op=OP.is_gt)

            # valid-token mask (f/g gate folded into per-row scalars later)
            tvb = work.tile([P, W], BF16, tag="tvb", name=f"tvb_{h}")
            nc.vector.tensor_copy(tvb[:], tvt[:])
            if not sp_zero:
                tva = work.tile([P, W], BF16, tag="tva", name=f"tva_{h}")
                nc.vector.tensor_mul(tva[:], tvb[:], act_bc[:])
                tvb = tva
            vop = work.tile([P, W], BF16, tag="vop", name=f"vop_{h}")
            nc.vector.tensor_mul(vop[:], tvb[:], is_op[:])

            # mpre[j] = running max of vop (per 256-tile)
            mpre = work.tile([P, W], BF16, tag="mpre", name=f"mpre_{h}")
            for t in range(TPH):
                ts = slice(t * S, (t + 1) * S)
                nc.vector.tensor_tensor_scan(
                    mpre[:][:, ts], vop[:][:, ts], vop[:][:, ts], 0.0,
                    op0=OP.max, op1=OP.max)

            # nsh[j] = 1 if no valid op strictly before j (within tile)
            nsh = work.tile([P, W], BF16, tag="nsh", name=f"nsh_{h}")
            nc.vector.tensor_single_scalar(
                nsh[:][:, 1:W], mpre[:][:, 0 : W - 1], 1.0, op=OP.not_equal)
            nc.vector.memset(nsh[:][:, 0::S], 1.0)
            ohi = work.tile([P, W], BF16, tag="ohi", name=f"ohi_{h}")
            nc.vector.tensor_mul(ohi[:], nsh[:], vop[:])

            # numbers before i*: tvb & ~mpre
            nm = work.tile([P, W], BF16, tag="nm", name=f"nm_{h}")
            nc.vector.tensor_single_scalar(nm[:], mpre[:], 1.0, op=OP.not_equal)
            vnum = work.tile([P, W], BF16, tag="vnum", name=f"vnum_{h}")
            nc.vector.tensor_mul(vnum[:], nm[:], tvb[:])

            # csuf[j] = inclusive suffix count of vnum (reverse scan per tile)
            csuf = work.tile([P, W], BF16, tag="csuf", name=f"csuf_{h}")
            for t in range(TPH):
                ts = slice(t * S, (t + 1) * S)
                vr = vnum[:][:, ts][:, ::-1]
                nc.vector.tensor_tensor_scan(
                    csuf[:][:, ts][:, ::-1], vr, vr, 0.0, op0=OP.add, op1=OP.max)

            # last & second-to-last number one-hots (constant compares)
            e0 = work.tile([P, W], BF16, tag="e0", name=f"e0_{h}")
            nc.vector.tensor_single_scalar(e0[:], csuf[:], 1.0, op=OP.is_equal)
            ohp0 = work.tile([P, W], BF16, tag="ohp0", name=f"ohp0_{h}")
            nc.vector.tensor_mul(ohp0[:], e0[:], vnum[:])
            e1 = work.tile([P, W], BF16, tag="e1", name=f"e1_{h}")
            nc.vector.tensor_single_scalar(e1[:], csuf[:], 2.0, op=OP.is_equal)
            ohp1 = work.tile([P, W], BF16, tag="ohp1", name=f"ohp1_{h}")
            nc.vector.tensor_mul(ohp1[:], e1[:], vnum[:])

            # per-row scalars for this half
            met2 = mpre[:][:, S - 1 :: S]       # [P, TPH] bf16
            total2 = csuf[:][:, 0::S]           # [P, TPH] bf16
            gate2 = gate[:, h * TPH : (h + 1) * TPH]
            acols = slice(h * TPH, (h + 1) * TPH)

            cnt1 = sm.tile([P, TPH], F32, name=f"cnt1_{h}")
            nc.gpsimd.tensor_scalar(cnt1[:], total2, 0.5, None, op0=OP.is_gt)
            cnt2 = sm.tile([P, TPH], F32, name=f"cnt2_{h}")
            nc.gpsimd.tensor_scalar(cnt2[:], total2, 1.5, None, op0=OP.is_gt)
            metg = sm.tile([P, TPH], F32, name=f"metg_{h}")
            nc.gpsimd.tensor_mul(metg[:], met2, gate2)
            fire2 = fire_a[:, acols]
            nc.gpsimd.tensor_mul(fire2, metg[:], cnt2[:])

            # masked-sum gathers (exact: at most one nonzero term)
            def gather_dve(dst_col, src_ap, mask_ap):
                scr = gsc.tile([P, S], F32, tag="gscr", name="gscr")
                nc.vector.scalar_tensor_tensor(
                    scr[:], src_ap, 0.0, mask_ap, op0=OP.bypass, op1=OP.mult,
                    accum_out=dst_col)

            def gather_act(dst_col, src_ap, mask_ap):
                scr = gsc.tile([P, S], F32, tag="gscp", name="gscp")
                nc.gpsimd.tensor_mul(scr[:], src_ap, mask_ap)
                scr2 = gsc.tile([P, S], F32, tag="gscq", name="gscq")
                nc.scalar.activation(scr2[:], scr[:], ACTF.Copy,
                                     accum_out=dst_col)

            for t in range(TPH):
                ts = slice(t * S, (t + 1) * S)
                col = slice(h * TPH + t, h * TPH + t + 1)
                tds = tdt[:][:, ts]
                ohp0s, ohp1s, ohis = ohp0[:][:, ts], ohp1[:][:, ts], ohi[:][:, ts]
                gather_dve(h0_a[:, col], tds, ohp0s)
                gather_dve(h1_a[:, col], tds, ohp1s)
                gather_act(vmax_a[:, col], m6[:][:, ts], ohis)
                gather_dve(vc_a[0][:, col], chs[2][:][:, ts], ohis)
                gather_dve(vc_a[1][:, col], chs[3][:][:, ts], ohis)
                gather_act(vc_a[2][:, col], chs[4][:][:, ts], ohis)
                gather_act(vc_a[3][:, col], chs[5][:][:, ts], ohis)
                gather_act(vc_a[4][:, col], chs[6][:][:, ts], ohis)

            # iv / iff (gps smalls)
            nmet = sm.tile([P, TPH], F32, name=f"nmet_{h}")
            nc.gpsimd.tensor_scalar(nmet[:], metg[:], -1.0, 1.0,
                                    op0=OP.mult, op1=OP.add)
            g2 = val_t[:, acols]
            nmg = sm.tile([P, TPH], F32, name=f"nmg_{h}")
            nc.gpsimd.tensor_mul(nmg[:], nmet[:], g2)
            nc.gpsimd.tensor_add(iffiv[:, NT + h * TPH : NT + (h + 1) * TPH],
                                 fire2, nmg[:])
            ncnt2 = sm.tile([P, TPH], F32, name=f"ncnt2_{h}")
            nc.gpsimd.tensor_scalar(ncnt2[:], cnt2[:], -1.0, 1.0,
                                    op0=OP.mult, op1=OP.add)
            q1 = sm.tile([P, TPH], F32, name=f"q1_{h}")
            nc.gpsimd.tensor_mul(q1[:], nmg[:], cnt1[:])
            ifu = sm.tile([P, TPH], F32, name=f"ifu_{h}")
            nc.gpsimd.tensor_mul(ifu[:], q1[:], ncnt2[:])
            q2 = sm.tile([P, TPH], F32, name=f"q2_{h}")
            nc.gpsimd.tensor_mul(q2[:], omf[:, acols], ifu[:])
            nc.gpsimd.tensor_add(iffiv[:, h * TPH : (h + 1) * TPH],
                                 q2[:], fin_t[:, acols])

            # tv[p1] <- 0, tv[i*] <- 0 where fire  (doesn't need r)
            u1 = work.tile([P, W], BF16, tag="u1", name=f"u1_{h}")
            nc.gpsimd.tensor_add(u1[:], ohp1[:], ohi[:])
            for t in range(TPH):
                ts = slice(t * S, (t + 1) * S)
                fcol = fire_a[:, h * TPH + t : h * TPH + t + 1]
                nw1 = gsc.tile([P, S], BF16, tag="nw1", name="nw1")
                nc.vector.tensor_scalar(nw1[:], u1[:][:, ts], fcol, 1.0,
                                        op0=OP.mult, op1=OP.not_equal)
                nc.vector.tensor_mul(tvt[:][:, ts], tvt[:][:, ts], nw1[:])
            nc.sync.dma_start(
                tvo_d[rows, :].rearrange("(t p) s -> p t s", p=P),
                tvt[:].rearrange("p (t s) -> p t s", t=TPH))

            half_state.append((tdt, ohp0, rows))

        # ---- batched r computation ([P, NT], one Ln + one Exp table load)
        radd = sm.tile([P, NT], F32)
        nc.gpsimd.tensor_add(radd[:], h1_a[:], h0_a[:])
        rsub = sm.tile([P, NT], F32)
        nc.gpsimd.tensor_sub(rsub[:], h1_a[:], h0_a[:])
        rmul = sm.tile([P, NT], F32)
        nc.gpsimd.tensor_mul(rmul[:], h1_a[:], h0_a[:])
        den = sm.tile([P, NT], F32)
        nc.gpsimd.tensor_scalar_add(den[:], h0_a[:], 1e-7)
        rec = sm.tile([P, NT], F32)
        nc.vector.reciprocal(rec[:], den[:])
        rdiv = sm.tile([P, NT], F32)
        nc.gpsimd.tensor_mul(rdiv[:], h1_a[:], rec[:])
        base = sm.tile([P, NT], F32)
        nc.gpsimd.tensor_scalar_max(base[:], h1_a[:], 1e-7)
        lg = sm.tile([P, NT], F32)
        nc.scalar.activation(lg[:], base[:], ACTF.Ln)
        pm = sm.tile([P, NT], F32)
        nc.gpsimd.tensor_mul(pm[:], lg[:], h0_a[:])
        rpow = sm.tile([P, NT], F32)
        nc.scalar.activation(rpow[:], pm[:], ACTF.Exp)

        nc.vector.memset(r_a[:], 0.0)
        ohc = sm.tile([P, NT], U8, name="ohc", bufs=2)
        for vc, res in zip(vc_a, [radd, rsub, rmul, rdiv, rpow]):
            nc.vector.tensor_tensor(ohc[:], vc[:], vmax_a[:], op=OP.is_equal)
            nc.vector.copy_predicated(r_a[:], ohc[:], res[:])

        nc.sync.dma_start(io_d.ap().rearrange("(p q) -> p q", p=P), iffiv[:])

        # ---- td scatter (needs r) + store
        for h, (tdt, ohp0, rows) in enumerate(half_state):
            for t in range(TPH):
                ts = slice(t * S, (t + 1) * S)
                cidx = h * TPH + t
                fcol = fire_a[:, cidx : cidx + 1]
                rcol = r_a[:, cidx : cidx + 1]
                w0 = gsc.tile([P, S], U8, tag="w0", name="w0")
                nc.vector.tensor_scalar(w0[:], ohp0[:][:, ts], fcol, None,
                                        op0=OP.mult)
                rb = gsc.tile([P, S], F32, tag="rb", name="rb")
                nc.vector.tensor_scalar(rb[:], zero_bc[:], 0.0, rcol,
                                        op0=OP.mult, op1=OP.add)
                nc.vector.copy_predicated(tdt[:][:, ts], w0[:], rb[:])
            nc.sync.dma_start(
                tdo_d[rows, :].rearrange("(t p) s -> p t s", p=P),
                tdt[:].rearrange("p (t s) -> p t s", t=TPH))

    nc.compile()
    return nc


_NC_CACHE = {}


def _get_nc(sp_zero: bool = True):
    if sp_zero not in _NC_CACHE:
        _NC_CACHE[sp_zero] = _build_nc(sp_zero)
    return _NC_CACHE[sp_zero]


def _make_in_maps(trans_valid, trans_dense, trans_op, if_finished, if_valid,
                  start_pos):
    tv = np.ascontiguousarray(np.asarray(trans_valid, np.float32))
    td = np.ascontiguousarray(np.asarray(trans_dense, np.float32))
    # layout-only: channel planes [7, B, S]
    op = np.ascontiguousarray(
        np.asarray(trans_op, np.float32).transpose(2, 0, 1))
    fin = np.asarray(if_finished, np.float32)
    val = np.asarray(if_valid, np.float32)
    sp = int(start_pos)
    act2 = np.ascontiguousarray(
        np.tile((np.arange(S) >= sp).astype(np.float32), TPH))
    in_maps = []
    for c in range(NCORES):
        rows = slice(c * BS, (c + 1) * BS)
        # fv[p, t] = fin[t*128+p] ; fv[p, NT+t] = val[t*128+p]
        fvc = np.concatenate(
            [fin[rows].reshape(NT, P).T, val[rows].reshape(NT, P).T], axis=1)
        in_maps.append({
            "tv": tv[rows], "td": td[rows],
            "op": np.ascontiguousarray(op[:, rows, :]),
            "fv": np.ascontiguousarray(fvc), "act2": act2,
        })
    return in_maps


def _unpack_outs(outs, trans_op):
    tv_out = np.concatenate([outs[c]["tv_out"] for c in range(NCORES)], axis=0)
    td_out = np.concatenate([outs[c]["td_out"] for c in range(NCORES)], axis=0)
    iff = np.empty(B, np.float32)
    iv = np.empty(B, np.float32)
    for c in range(NCORES):
        arr = outs[c]["iffiv"].reshape(P, 2 * NT)
        rows = slice(c * BS, (c + 1) * BS)
        iff[rows] = arr[:, 0:NT].T.reshape(BS)
        iv[rows] = arr[:, NT : 2 * NT].T.reshape(BS)
    return tv_out, td_out, np.asarray(trans_op, np.float32), iff, iv


def kernel(trans_valid, trans_dense, trans_op, if_finished, if_valid, start_pos):
    nc = _get_nc(int(start_pos) == 0)
    in_maps = _make_in_maps(trans_valid, trans_dense, trans_op, if_finished,
                            if_valid, start_pos)
    res = run_bass_kernel_spmd(nc, in_maps, core_ids=list(range(NCORES)))
    return _unpack_outs(res.results, trans_op)
